# revision 33
# baseline (speedup 1.0000x reference)
"""Trainium2 Bass kernel for nn_Attention (7-label multi-head attention with
softmax over head_dim), data-parallel over 8 NeuronCores.

Math (per batch row b):
  K = key @ wk_w + wk_b            [B, 300]
  V = value @ wv_w + wv_b          [B, 300]
  Q = query @ wq_w + wq_b          [7, 300]
  e[b,l,h,d]   = Q[l,h,d] * K[b,h,d] / sqrt(50)
  attn         = softmax over d (within each 50-wide head segment)
  x[b,l,h,d]   = attn * V[b,h,d]
  out[b,l,:]   = x[b,l,:] @ fc_w + fc_b

Device layout: everything downstream of the projections is computed in
"transposed" layout (hidden dim on partitions, batch on the free axis), so
the only transposes needed are of the key/value inputs (PE transpose of
bf16 tiles), and the fc matmul consumes x directly as the stationary
operand, producing natural-layout output rows.
"""

import math
import sys

for _p in ("/opt/trn_rl_repo",):
    if _p not in sys.path:
        sys.path.insert(0, _p)

import numpy as np
import ml_dtypes

import concourse.bass as bass
import concourse.tile as tile
from concourse import bacc, mybir

BF16 = ml_dtypes.bfloat16

N_CORES = 8
B_FULL = 16384
IMG = 2048
HID = 300
NH = 6
HD = 50
NLAB = 7
SCALE = float(np.sqrt(HD))

BC = B_FULL // N_CORES          # 2048 batch rows per core
KCH = IMG // 128                # 16 contraction chunks for the projections
CHUNKS = [(0, 128), (128, 256), (256, 300)]   # hidden-dim partition chunks
NCH = len(CHUNKS)


def _seg(hd):
    return hd // HD             # head index of a hidden position


def host_constants():
    """Small replicated constants derived from nothing (pure structure)."""
    # Segment-sum indicator: lhsT for s[6l+h, b] += sum_d p_l[h*50+d, b]
    ind_seg = np.zeros((128, NCH, NLAB, NLAB * NH), dtype=np.float32)
    # Broadcast indicator (transpose): r_rep[hd, b] = r[6l + seg(hd), b]
    ind_bc = np.zeros((NLAB * NH, NCH, NLAB, 128), dtype=np.float32)
    for c, (c0, c1) in enumerate(CHUNKS):
        for p in range(c1 - c0):
            h = _seg(c0 + p)
            for l in range(NLAB):
                ind_seg[p, c, l, NH * l + h] = 1.0
                ind_bc[NH * l + h, c, l, p] = 1.0
    ident = np.eye(128, dtype=np.float32)
    xpad = np.zeros((21, 3584), dtype=np.float32)
    xpad[20, :] = 1.0
    return {
        "ind_seg": ind_seg.astype(BF16),
        "ind_bc": ind_bc,                       # f32 (used in f32r matmul)
        "ident_bf16": ident.astype(BF16),
        "ident_f32": ident,
        "xpad": xpad,
    }


def build_body(tc, io, bc, nb, dbg=None):
    """Emit the per-core program. io: dict name -> bass.AP (DRAM)."""
    nc = tc.nc

    def dump(name, ap):
        if dbg is not None and name in dbg:
            if ap.dtype == mybir.dt.float32r:
                ap = ap.bitcast(mybir.dt.float32)
            nc.sync.dma_start(dbg[name], ap)
    f32 = mybir.dt.float32
    f32r = mybir.dt.float32r
    bf16 = mybir.dt.bfloat16
    AF = mybir.ActivationFunctionType
    ALU = mybir.AluOpType

    nblk = bc // nb
    sub = nb // 2                    # x / fc sub-block granularity
    assert (nb * NLAB) % 128 == 0 and (sub * NLAB) % 128 == 0
    mch_per_sub = sub * NLAB // 128  # fc output row-chunks per sub-block

    key = io["key"]                  # [bc, IMG] bf16
    value = io["value"]              # [bc, IMG] bf16
    out = io["out"]                  # [bc*7, 300] f32

    # ---------------- pools ----------------
    ctx = tc.ctx  # ExitStack provided by caller wrapper
    singles = ctx.enter_context(tc.tile_pool(name="singles", bufs=1))
    natp = ctx.enter_context(tc.tile_pool(name="natp", bufs=3))
    ktp = ctx.enter_context(tc.tile_pool(name="ktp", bufs=1))
    kvsb = ctx.enter_context(tc.tile_pool(name="kvsb", bufs=2))
    pp = ctx.enter_context(tc.tile_pool(name="pp", bufs=22))
    tt = ctx.enter_context(tc.tile_pool(name="tt", bufs=6))
    xx = ctx.enter_context(tc.tile_pool(name="xx", bufs=1))
    srp = ctx.enter_context(tc.tile_pool(name="srp", bufs=2))
    outp = ctx.enter_context(tc.tile_pool(name="outp", bufs=4))
    ps_tr = ctx.enter_context(tc.tile_pool(name="ps_tr", bufs=2, space="PSUM"))
    ps_pj = ctx.enter_context(tc.tile_pool(name="ps_pj", bufs=3, space="PSUM"))
    ps_sc = ctx.enter_context(tc.tile_pool(name="ps_sc", bufs=2, space="PSUM"))

    # ---------------- constants / weights ----------------
    ident_b = singles.tile([128, 128], bf16)
    nc.sync.dma_start(ident_b, io["ident_bf16"])
    ident_f = singles.tile([128, 128], f32)
    nc.sync.dma_start(ident_f, io["ident_f32"])
    ind_seg = singles.tile([128, NCH, NLAB, NLAB * NH], bf16)
    nc.sync.dma_start(ind_seg, io["ind_seg"])
    ind_bc = singles.tile([NLAB * NH, NCH, NLAB, 128], f32r)
    nc.sync.dma_start(ind_bc, io["ind_bc"])

    # projection weights, bf16, one tile per contraction chunk [128, 300]
    wk_sb = []
    wv_sb = []
    for k in range(KCH):
        wkt = singles.tile([128, HID], bf16, tag=f"wk{k}")
        nc.sync.dma_start(wkt, io["wk_w"][k * 128:(k + 1) * 128, :])
        wk_sb.append(wkt)
        wvt = singles.tile([128, HID], bf16, tag=f"wv{k}")
        nc.sync.dma_start(wvt, io["wv_w"][k * 128:(k + 1) * 128, :])
        wv_sb.append(wvt)

    # fc weights (f32). Last chunk is padded to 65 rows: rows 0:44 =
    # fc_w[256:300], rows 44:64 = 0, row 64 = fc_b (matching the x tile
    # whose row 64 is a constant-ones row -> fc bias comes out of the
    # matmul for free).
    fcw_sb = []
    for c, (c0, c1) in enumerate(CHUNKS):
        ext = 21 if c == NCH - 1 else 0
        t = singles.tile([c1 - c0 + ext, HID], f32r, tag=f"fcw{c}")
        nc.sync.dma_start(t, io["fc_w_ext"][c0:c1 + ext, :])
        fcw_sb.append(t)

    # bias columns [300,1] -> per-chunk [csz,1]
    def load_col(name):
        tiles = []
        for c, (c0, c1) in enumerate(CHUNKS):
            t = singles.tile([c1 - c0, 1], f32, tag=f"{name}{c}")
            nc.sync.dma_start(t, io[name][c0:c1, :])
            tiles.append(t)
        return tiles

    wqb_sb = load_col("wq_b")
    wkb_sb = load_col("wk_b")
    wvb_sb = load_col("wv_b")

    # ---------------- Q preparation (tiny) ----------------
    # queryT chunks [csz, 7], then QT = wq_w.T-contract, Qs = (QT+b)/SCALE
    query_sb = singles.tile([NLAB, HID], f32)
    nc.sync.dma_start(query_sb, io["query"])
    wqw_sb = []
    for m in range(NCH):
        m0, m1 = CHUNKS[m]
        t = singles.tile([m1 - m0, HID], f32, tag=f"wqw{m}")
        nc.sync.dma_start(t, io["wq_w"][m0:m1, :])
        wqw_sb.append(t)

    qt_sb = []
    for c, (c0, c1) in enumerate(CHUNKS):
        csz = c1 - c0
        pt = ps_sc.tile([128, 512], f32, tag="scr")
        nc.tensor.transpose(pt[:csz, :NLAB], query_sb[:, c0:c1], ident_f[:NLAB, :NLAB])
        t = singles.tile([csz, NLAB], f32, tag=f"qt{c}")
        nc.vector.tensor_copy(t, pt[:csz, :NLAB])
        qt_sb.append(t)

    qs_sb = []   # Qs^T per chunk [csz, 7]  (scale for exp)
    bs_sb = []   # (Qs*wk_b)^T per chunk [csz, 7] (bias for exp)
    for c, (c0, c1) in enumerate(CHUNKS):
        csz = c1 - c0
        pq = ps_sc.tile([128, 512], f32, tag="scr")
        for m in range(NCH):
            m0, m1 = CHUNKS[m]
            nc.tensor.matmul(
                pq[:csz, :NLAB],
                wqw_sb[m][:, c0:c1],
                qt_sb[m],
                start=(m == 0), stop=(m == NCH - 1),
            )
        qs = singles.tile([csz, NLAB], f32, tag=f"qs{c}")
        nc.vector.tensor_scalar(
            qs, pq[:csz, :NLAB], wqb_sb[c], 1.0 / SCALE, op0=ALU.add, op1=ALU.mult
        )
        qs_sb.append(qs)
        bs = singles.tile([csz, NLAB], f32, tag=f"bs{c}")
        nc.vector.tensor_scalar(bs, qs, wkb_sb[c], None, op0=ALU.mult)
        bs_sb.append(bs)

    # ---------------- main loop over batch blocks ----------------
    for n in range(nblk):
        b0 = n * nb
        # -- PE transposes -> keyT/valueT [128, KCH, nb] bf16 --
        # j-outer so only one natural-layout row tile is live at a time.
        kT = ktp.tile([128, KCH, nb], bf16, tag="kT")
        vT = ktp.tile([128, KCH, nb], bf16, tag="vT")
        for j in range(nb // 128):
            for (src, dst, tg) in ((key, kT, "natk"), (value, vT, "natv")):
                for half in range(2):   # 8 k-chunks (one PSUM bank) per half
                    kc8 = half * 8
                    nat = natp.tile([128, 1024], bf16, tag=tg, name="nat")
                    nc.sync.dma_start(
                        nat,
                        src[b0 + j * 128: b0 + (j + 1) * 128,
                            kc8 * 128:(kc8 + 8) * 128],
                    )
                    pt = ps_tr.tile([128, 8, 128], bf16, tag="tr")
                    for dk in range(8):
                        nc.tensor.transpose(
                            pt[:, dk, :], nat[:, dk * 128:(dk + 1) * 128],
                            ident_b,
                        )
                    nc.vector.tensor_copy(
                        dst[:, kc8:kc8 + 8, j * 128:(j + 1) * 128], pt
                    )

        # -- projections: two waves (K then V) of 3 psum chunks each --
        kt_sb = []   # K^T chunks [csz, nb] f32 (no bias; folded into exp)
        vt_sb = []   # (V^T + wv_b) chunks [csz, nb] f32
        for (w_sb, xT, dst_list, bias) in (
            (wk_sb, kT, kt_sb, None),
            (wv_sb, vT, vt_sb, wvb_sb),
        ):
            for c, (c0, c1) in enumerate(CHUNKS):
                csz = c1 - c0
                ppj = ps_pj.tile([128, nb], f32, tag="proj")
                for k in range(KCH):
                    nc.tensor.matmul(
                        ppj[:csz, :],
                        w_sb[k][:, c0:c1],
                        xT[:, k, :],
                        start=(k == 0), stop=(k == KCH - 1),
                    )
                t = kvsb.tile([csz, nb], f32, tag=f"kv{c}")
                if bias is None:
                    nc.vector.tensor_copy(t, ppj[:csz, :])
                else:
                    nc.vector.tensor_scalar_add(t, ppj[:csz, :], bias[c])
                if n == 0:
                    dump(f"{'kt' if bias is None else 'vt'}{c}", t)
                dst_list.append(t)

        # -- exp: p[c][l] = exp(Kt*qs + bs)  (bf16) --
        p_t = {}
        for l in range(NLAB):
            for c, (c0, c1) in enumerate(CHUNKS):
                csz = c1 - c0
                pt = pp.tile([csz, nb], bf16, tag="p")
                nc.scalar.activation(
                    pt, kt_sb[c], AF.Exp,
                    bias=bs_sb[c][:, l:l + 1], scale=qs_sb[c][:, l:l + 1],
                )
                p_t[(c, l)] = pt

        # -- segment sums s[42, nb] via indicator matmuls (bf16, accumulate) --
        s_ps = ps_sc.tile([128, nb], f32, tag="scr")
        first = True
        for l in range(NLAB):
            for c, (c0, c1) in enumerate(CHUNKS):
                csz = c1 - c0
                nc.tensor.matmul(
                    s_ps[:NLAB * NH, :],
                    ind_seg[:csz, c, l, :],
                    p_t[(c, l)],
                    start=first, stop=(l == NLAB - 1 and c == NCH - 1),
                )
                first = False
        s_sb = srp.tile([NLAB * NH, nb], f32, tag="s")
        nc.vector.tensor_copy(s_sb, s_ps[:NLAB * NH, :])
        r_sb = srp.tile([NLAB * NH, nb], f32r, tag="r")
        with nc.allow_low_precision(reason="f32r output for PE broadcast"):
            nc.vector.reciprocal(r_sb, s_sb)
        if n == 0:
            dump("s", s_sb)
            dump("p00", p_t[(0, 0)])

        # -- t = p * Vt (gpsimd), x = t * r_rep (vector, r_rep from PSUM) --
        x_t = [
            [xx.tile([c1 - c0 + (21 if c == NCH - 1 else 0), sub * NLAB], f32r,
                     tag=f"x{c}s{s}", name=f"x{c}s{s}")
             for c, (c0, c1) in enumerate(CHUNKS)]
            for s in range(nb // sub)
        ]
        for s in range(nb // sub):
            # rows 44:64 zero (never written otherwise), row 64 = ones;
            # written via broadcast DMA from a tiny f32r constant (memset
            # on f32r tiles fails walrus ISA checks).
            nc.sync.dma_start(x_t[s][NCH - 1][44:65, :], io["xpad"][:, :sub * NLAB])

        for l in range(NLAB):
            for c, (c0, c1) in enumerate(CHUNKS):
                csz = c1 - c0
                tmul = tt.tile([csz, nb], f32, tag="t")
                nc.gpsimd.tensor_mul(tmul, p_t[(c, l)], vt_sb[c])
                rr = ps_sc.tile([128, nb], f32, tag="scr")
                nc.tensor.matmul(
                    rr[:csz, :],
                    ind_bc[:, c, l, :csz],
                    r_sb,
                    start=True, stop=True,
                )
                for s in range(nb // sub):
                    # x view: columns b*NLAB + l for b in [s*sub, (s+1)*sub)
                    xv = x_t[s][c][:csz, :].rearrange(
                        "p (b l7) -> p b l7", l7=NLAB
                    )[:, :, l]
                    nc.vector.tensor_mul(
                        xv, tmul[:, s * sub:(s + 1) * sub],
                        rr[:csz, s * sub:(s + 1) * sub],
                    )

        if n == 0:
            dump("x0", x_t[0][0])
            dump("x1", x_t[0][1])
            dump("x2", x_t[0][2])

        # -- fc: out rows = x^T chunks (stationary) @ fc_w_ext --
        for s in range(nb // sub):
            for m in range(mch_per_sub):
                po = ps_sc.tile([128, HID], f32, tag="scr")
                for c, (c0, c1) in enumerate(CHUNKS):
                    nc.tensor.matmul(
                        po[:, :HID],
                        x_t[s][c][:, m * 128:(m + 1) * 128],
                        fcw_sb[c],
                        start=(c == 0), stop=(c == NCH - 1),
                    )
                o_sb = outp.tile([128, HID], f32, tag="o")
                nc.scalar.copy(o_sb, po[:, :HID])
                if n == 0 and s == 0 and m == 0:
                    dump("o0", o_sb)
                row0 = (b0 + s * sub) * NLAB + m * 128
                nc.sync.dma_start(out[row0:row0 + 128, :], o_sb)


class _Built:
    def __init__(self, bc, nb, debug=False):
        self.bc = bc
        self.nb = nb
        nc = bacc.Bacc(
            "TRN2", target_bir_lowering=False, debug=False,
            enable_asserts=False, num_devices=N_CORES,
        )
        f32 = mybir.dt.float32
        f32r = mybir.dt.float32r
        bf16 = mybir.dt.bfloat16
        io = {}

        def d(name, shape, dt):
            io[name] = nc.dram_tensor(name, shape, dt, kind="ExternalInput").ap()

        d("key", [bc, IMG], bf16)
        d("value", [bc, IMG], bf16)
        d("query", [NLAB, HID], f32)
        d("wq_w", [HID, HID], f32)
        d("wk_w", [IMG, HID], bf16)
        d("wv_w", [IMG, HID], bf16)
        d("fc_w_ext", [HID + 21, HID], f32r)
        d("wq_b", [HID, 1], f32)
        d("wk_b", [HID, 1], f32)
        d("wv_b", [HID, 1], f32)
        d("xpad", [21, 3584], f32r)
        d("ind_seg", [128, NCH, NLAB, NLAB * NH], bf16)
        d("ind_bc", [NLAB * NH, NCH, NLAB, 128], f32r)
        d("ident_bf16", [128, 128], bf16)
        d("ident_f32", [128, 128], f32)
        io["out"] = nc.dram_tensor(
            "out", [bc * NLAB, HID], f32, kind="ExternalOutput"
        ).ap()

        dbg = None
        if debug:
            dbg = {}
            sub = nb // 2
            for name, shape in [
                ("kt0", [128, nb]), ("kt1", [128, nb]), ("kt2", [44, nb]),
                ("vt0", [128, nb]), ("vt1", [128, nb]), ("vt2", [44, nb]),
                ("s", [NLAB * NH, nb]), ("p00", [128, nb]),
                ("x0", [128, sub * NLAB]), ("x1", [128, sub * NLAB]),
                ("x2", [65, sub * NLAB]), ("o0", [128, HID]),
            ]:
                dt = mybir.dt.bfloat16 if name == "p00" else f32
                dbg[name] = nc.dram_tensor(
                    f"dbg_{name}", shape, dt, kind="ExternalOutput"
                ).ap()

        import contextlib
        with tile.TileContext(nc) as tc:
            with contextlib.ExitStack() as stack:
                tc.ctx = stack
                build_body(tc, io, bc, nb, dbg=dbg)
        nc.compile()
        self.nc = nc


_BUILT = {}


def _get_built(bc=BC, nb=512):
    key = (bc, nb)
    if key not in _BUILT:
        _BUILT[key] = _Built(bc, nb)
    return _BUILT[key]


def make_in_maps(inputs, bc=BC):
    """Shard FULL inputs -> list of per-core input dicts."""
    consts = host_constants()
    fc_w_ext = np.concatenate(
        [
            np.asarray(inputs["fc_w"], dtype=np.float32),
            np.zeros((20, HID), dtype=np.float32),
            np.asarray(inputs["fc_b"], dtype=np.float32).reshape(1, HID),
        ],
        axis=0,
    )
    common = {
        "query": np.ascontiguousarray(inputs["query"], dtype=np.float32),
        "wq_w": np.ascontiguousarray(inputs["wq_w"], dtype=np.float32),
        "wk_w": np.ascontiguousarray(inputs["wk_w"]).astype(BF16),
        "wv_w": np.ascontiguousarray(inputs["wv_w"]).astype(BF16),
        "fc_w_ext": fc_w_ext,
        "wq_b": np.ascontiguousarray(inputs["wq_b"], np.float32).reshape(HID, 1),
        "wk_b": np.ascontiguousarray(inputs["wk_b"], np.float32).reshape(HID, 1),
        "wv_b": np.ascontiguousarray(inputs["wv_b"], np.float32).reshape(HID, 1),
        **consts,
    }
    key_b = np.ascontiguousarray(inputs["key"]).astype(BF16)
    val_b = np.ascontiguousarray(inputs["value"]).astype(BF16)
    in_maps = []
    for i in range(N_CORES):
        m = dict(common)
        m["key"] = key_b[i * bc:(i + 1) * bc]
        m["value"] = val_b[i * bc:(i + 1) * bc]
        in_maps.append(m)
    return in_maps


def kernel(**inputs) -> np.ndarray:
    from concourse.bass_utils import run_bass_kernel_spmd

    built = _get_built()
    in_maps = make_in_maps(inputs)
    res = run_bass_kernel_spmd(built.nc, in_maps, list(range(N_CORES)))
    outs = [
        np.asarray(res.results[i]["out"], dtype=np.float32).reshape(BC, NLAB, HID)
        for i in range(N_CORES)
    ]
    return np.concatenate(outs, axis=0)
